# revision 16
# baseline (speedup 1.0000x reference)
"""ConsecutiveLoss (L1) Trainium2 kernel.

Reference semantics (per full input x [4096, 8192] f32):
    rl[i]     = count_nonzero(x[i, :])
    per_row_i = sum_{j=0}^{8190} |x[i,j+1]-x[i,j]| * (j < rl[i]-1) / rl[i]
    out       = sum_{i>=1} per_row_i / 4096

Sharding: 4096 rows split across 8 NeuronCores (512 rows each). Each core
computes per-row losses for its rows; host gathers and does the final
(4095-element) reduction.

Per-core kernel: 4 tiles of [128 rows x 8192], two column-chunks each for
pipelining. Per tile:
  - DMA the tile into SBUF (two 2 MiB chunks)
  - rl: DVE tensor_scalar(not_equal)+accum per chunk (2x single-src mode)
  - sub: DVE tensor_tensor(subtract), bf16 out
  - abs: ACT activation(Abs) bf16
  - masked row-sum: DVE scalar_tensor_tensor
        (iota16 is_lt rl-1) mult |d|, accum_out -> rowsum
    with iota int16 + |d| bf16 (16-bit streams for a shot at 2x mode)
  - per-row loss = (rs0+rs1) * 1/rl; collected in SBUF, one DMA out.

This walrus build accepts only ONE sync wait per ISA instruction; TileContext
emits multi-wait instructions (stage-1B consumers + the tail drain). Both are
patched below by splitting waits onto single-wait NoOp/Drain carriers.
"""

import os
from operator import add

import numpy as np

import concourse.bass as bass
import concourse.mybir as mybir
import concourse.tile as tile
from concourse.bass_utils import run_bass_kernel_spmd

# --- workaround: single-sync-wait-per-instruction walrus ---
_ORIG_DRAIN_AND_BARRIER = tile.TileContext._drain_and_barrier


def _split_drain_and_barrier(self, tick_clock, wait_clock):
    from concourse.tile import ScopedClock

    drain_inst = self.nc.sync.drain()
    wait_clock.add_sem_waits(
        drain_inst.ins, ScopedClock({None: tick_clock.global_clock})
    )
    si = drain_inst.ins.sync_info
    waits = list(si.on_wait) if si is not None and si.on_wait else []
    if len(waits) > 1:
        ups = list(si.on_update) if si.on_update else []
        drain_inst.ins.sync_info = mybir.SyncInfo(on_wait=[waits[0]], on_update=ups)
        # spread the split waits across engines so they drain in parallel
        engs = [self.nc.tensor, self.nc.gpsimd, self.nc.vector,
                self.nc.scalar, self.nc.sync]
        for i, w in enumerate(waits[1:]):
            extra = engs[i % len(engs)].drain()
            extra.ins.sync_info = mybir.SyncInfo(on_wait=[w], on_update=[])

    self.nc.all_engine_barrier()
    assert self.sems is not None
    popped = self.nc._tile_sem_poison_stack.pop()
    assert popped is self._sem_poison
    self.nc.clear_and_free_semaphores(list(self.sems.allocated().values()))
    self.nc.all_engine_barrier()


tile.TileContext._drain_and_barrier = _split_drain_and_barrier

_ORIG_COMMIT = tile.TileContext._commit_instruction


def _split_commit(self, inst, lazy_reg_writes: bool = True):
    si = getattr(inst, "sync_info", None)
    if (
        si is not None
        and si.on_wait
        and len(si.on_wait) > 1
        and inst.engine != mybir.EngineType.Unassigned
    ):
        waits = list(si.on_wait)
        ups = list(si.on_update) if si.on_update else []
        for w in waits[:-1]:
            nop = mybir.InstNoOp(
                name=self.nc.get_next_instruction_name(),
                sync_info=mybir.SyncInfo(on_wait=[w], on_update=[]),
                bass_nofuse=True,
                engine=inst.engine,
                text_hint="wait_split",
            )
            _ORIG_COMMIT(self, nop, lazy_reg_writes=False)
        inst.sync_info = mybir.SyncInfo(on_wait=[waits[-1]], on_update=ups)
    return _ORIG_COMMIT(self, inst, lazy_reg_writes)


tile.TileContext._commit_instruction = _split_commit


def _audit_multi_waits(nc) -> list[str]:
    bad = []
    for name, ins in nc.inst_map.items():
        si = getattr(ins, "sync_info", None)
        if si is not None and si.on_wait and len(si.on_wait) > 1:
            bad.append(f"{type(ins).__name__} {name} {ins.engine} x{len(si.on_wait)}")
    return bad


N_CORES = 8
ROWS, COLS = 4096, 8192
SH_ROWS = ROWS // N_CORES  # 512 rows per core
P = 128                    # SBUF partitions
N_TILES = SH_ROWS // P     # 4 tiles per core
D = COLS - 1               # 8191 diffs per row
B = 4094                   # even sub-chunk boundary (keeps 16-bit APs 4B-aligned)
F32 = mybir.dt.float32
BF16 = mybir.dt.bfloat16
I16 = mybir.dt.int16


def build_nc_fast(reps: int = 1):
    """Zero-free specialization: rl == COLS for every row, so the nonzero
    count and the validity mask are degenerate. Per tile the only work is
    d = x[:,1:]-x[:,:-1] (DVE) and rowsum |d| (ACT Abs + accum), both of
    which pipeline under the x DMA — the kernel is DMA-bound.

    x DMAs alternate between the SP and PE (idle) queues so two hardware
    DMA rings pull concurrently. The last tile's chunks taper (2M/1M/
    512K/512K) so the serial compute tail after the final byte is short.
    """
    nc = bass.Bass("TRN2", target_bir_lowering=False, debug=False)
    x = nc.dram_tensor("x", [SH_ROWS, COLS], F32, kind="ExternalInput").ap()
    y = nc.dram_tensor("y", [P, N_TILES], F32, kind="ExternalOutput").ap()

    # Chunk sizes per tile, from an offline pipeline-simulator fit of the
    # measured rates (dve 1.08 ns/col, act 0.91 + per-op consts, dma
    # packet-size-aware: chunks under ~2048 cols transfer at reduced
    # bandwidth, so sizes stay in the 1.6k-2.8k band with a mild end taper).
    sizes = [
        [2048, 2048, 2048, 2048],
        [2688, 2688, 2816],
        [2048, 2048, 2048, 2048],
        [2304, 2176, 2048, 1664],
    ]
    assert all(sum(s) == COLS for s in sizes)

    def diffs(cols):
        return [
            (0 if i == 0 else cols[i] - 2, cols[i + 1] - 2 if i + 2 < len(cols) else D)
            for i in range(len(cols) - 1)
        ]

    def bounds(szs):
        cols = [0]
        for s in szs:
            cols.append(cols[-1] + s)
        return cols

    plans = [(bounds(s), diffs(bounds(s))) for s in sizes]
    dve_abs = set()  # (tile, chunk) -> abs on DVE; ISA rejects abs_max on DVE

    with tile.TileContext(nc) as tc:
        with (
            tc.tile_pool(name="xin", bufs=3) as xpool,
            tc.tile_pool(name="scr", bufs=2) as spool,
            tc.tile_pool(name="small", bufs=2) as smpool,
            tc.tile_pool(name="outp", bufs=1) as opool,
        ):
            loss = opool.tile([P, N_TILES], F32)
            for t in range(N_TILES * reps):
                t = t % N_TILES
                dma_cols, sub_chunks = plans[t]
                rows = slice(t * P, (t + 1) * P)
                xt = xpool.tile([P, COLS], F32, tag="xt")
                for c0, c1 in zip(dma_cols[:-1], dma_cols[1:]):
                    nc.sync.dma_start(xt[:, c0:c1], x[rows, c0:c1])
                nchunk = len(sub_chunks)
                # sbf bufs=4: DVE(t+1) must not serialize behind ACT(t)
                # draining sbf(t-1); abf is write-only junk -> one slot.
                sbf = spool.tile([P, COLS], BF16, tag="sbf", bufs=4)
                abf = spool.tile([P, COLS], BF16, tag="abf", bufs=1)
                rs = smpool.tile([P, nchunk], F32, tag="rs")
                rj = smpool.tile([P, nchunk], F32, tag="rj")
                for c, (j0, j1) in enumerate(sub_chunks):
                    nc.vector.tensor_tensor(
                        sbf[:, j0:j1],
                        xt[:, j0 + 1 : j1 + 1],
                        xt[:, j0:j1],
                        mybir.AluOpType.subtract,
                    )
                    if (t, c) in dve_abs:
                        # |d| = max(abs_max(d, 0), d); STT encodes with accum
                        dj = spool.tile([P, 2048], BF16, tag="dj", bufs=1)
                        nc.vector.scalar_tensor_tensor(
                            dj[:, : j1 - j0],
                            sbf[:, j0:j1],
                            0.0,
                            sbf[:, j0:j1],
                            mybir.AluOpType.abs_max,
                            mybir.AluOpType.max,
                            accum_out=rs[:, c : c + 1],
                        )
                    else:
                        nc.scalar.activation(
                            abf[:, j0:j1],
                            sbf[:, j0:j1],
                            mybir.ActivationFunctionType.Abs,
                            accum_out=rs[:, c : c + 1],
                        )
                nc.vector.tensor_scalar(
                    rj[:], rs[:], 0.0, 0.0,
                    mybir.AluOpType.add,
                    mybir.AluOpType.add,
                    accum_out=loss[:, t : t + 1],
                )
            nc.sync.dma_start(y[:, :], loss[:])
    bad = _audit_multi_waits(nc)
    if bad:
        raise RuntimeError(f"multi-wait instructions present: {bad}")
    return nc


def build_nc(variant: str | None = None, reps: int = 1):
    """Build the per-core Bass program (same program for all 8 cores).

    reps>1 repeats the whole body (same inputs/outputs) for dispatch-
    overhead-cancelling wall-clock benchmarking: HW ~= (T_r - T_1)/(r-1).
    """
    nc = bass.Bass("TRN2", target_bir_lowering=False, debug=False)
    x = nc.dram_tensor("x", [SH_ROWS, COLS], F32, kind="ExternalInput").ap()
    iota = nc.dram_tensor("iota16", [P, D], I16, kind="ExternalInput").ap()
    y = nc.dram_tensor("y", [P, 2 * N_TILES], F32, kind="ExternalOutput").ap()

    H = COLS // 2  # DMA/nz chunk size
    sub_chunks = [(0, B), (B, D)]  # diff index ranges

    with tile.TileContext(nc) as tc:
        with (
            tc.tile_pool(name="const", bufs=1) as cpool,
            tc.tile_pool(name="xin", bufs=2) as xpool,
            tc.tile_pool(name="scr", bufs=3) as spool,
            tc.tile_pool(name="small", bufs=2) as smpool,
            tc.tile_pool(name="outp", bufs=1) as opool,
        ):
            it16 = cpool.tile([P, D], I16)
            nc.sync.dma_start(it16[:], iota[:, :])
            loss = opool.tile([P, 2 * N_TILES], F32)
            for t in range(N_TILES * reps):
                t = t % N_TILES
                rows = slice(t * P, (t + 1) * P)
                xt = xpool.tile([P, COLS], F32, tag="xt")
                rlh = smpool.tile([P, 2], F32, tag="rlh")
                nzj = spool.tile([P, COLS], BF16, tag="big")
                for c in range(2):
                    cs = slice(c * H, (c + 1) * H)
                    nc.sync.dma_start(xt[:, cs], x[rows, cs])
                    # rl chunk count: accum((x != 0) + 0)
                    nc.vector.tensor_scalar(
                        nzj[:, cs],
                        xt[:, cs],
                        0.0,
                        0.0,
                        mybir.AluOpType.not_equal,
                        mybir.AluOpType.add,
                        accum_out=rlh[:, c : c + 1],
                    )
                # rl_m1 = (rlh0 - 1) + rlh1
                rl_m1 = smpool.tile([P, 1], F32, tag="rl_m1")
                nc.vector.scalar_tensor_tensor(
                    rl_m1[:],
                    rlh[:, 0:1],
                    -1.0,
                    rlh[:, 1:2],
                    mybir.AluOpType.add,
                    mybir.AluOpType.add,
                )
                sbf = spool.tile([P, COLS], BF16, tag="big")
                abf = spool.tile([P, COLS], BF16, tag="big")
                rs = smpool.tile([P, 2], F32, tag="rs")
                for (j0, j1) in sub_chunks:
                    # d = x[:, j+1] - x[:, j] for j in [j0, j1)
                    nc.vector.tensor_tensor(
                        sbf[:, j0:j1],
                        xt[:, j0 + 1 : j1 + 1],
                        xt[:, j0:j1],
                        mybir.AluOpType.subtract,
                    )
                    nc.scalar.activation(
                        abf[:, j0:j1],
                        sbf[:, j0:j1],
                        mybir.ActivationFunctionType.Abs,
                    )
                # masked row-sum chunks: (iota < rl-1) * |d|, accum
                # (junk `out` written in-place over abf)
                for c, (j0, j1) in enumerate(sub_chunks):
                    nc.vector.scalar_tensor_tensor(
                        abf[:, j0:j1],
                        it16[:, j0:j1],
                        rl_m1[:],
                        abf[:, j0:j1],
                        mybir.AluOpType.is_lt,
                        mybir.AluOpType.mult,
                        accum_out=rs[:, c : c + 1],
                    )
                # stage per-tile partial sums + rl-1; division happens on host
                nc.vector.tensor_tensor(
                    loss[:, 2 * t : 2 * t + 1], rs[:, 0:1], rs[:, 1:2],
                    mybir.AluOpType.add,
                )
                nc.vector.tensor_scalar(
                    loss[:, 2 * t + 1 : 2 * t + 2], rl_m1[:], 1.0, None,
                    mybir.AluOpType.add,
                )
            # y[p, 2t] = rowsum, y[p, 2t+1] = rl
            nc.sync.dma_start(y[:, :], loss[:])
    bad = _audit_multi_waits(nc)
    if bad:
        raise RuntimeError(f"multi-wait instructions present: {bad}")
    return nc


_NC_CACHE: dict[str, object] = {}


def _get_nc(variant: str | None = None):
    key = variant or os.environ.get("CONSEC_VARIANT", "fast")
    if key not in _NC_CACHE:
        if key == "fast":
            _NC_CACHE[key] = build_nc_fast()
        else:
            _NC_CACHE[key] = build_nc(key)
    return _NC_CACHE[key]


def _losses_from_y(y: np.ndarray) -> np.ndarray:
    """y [P, 2*N_TILES] -> per-row losses [SH_ROWS] (local row = t*P + p)."""
    y = y.reshape(P, N_TILES, 2)
    rs = y[:, :, 0].T.reshape(-1)   # [N_TILES*P] row-major by (t, p)
    rl = y[:, :, 1].T.reshape(-1)
    return (rs.astype(np.float32) / rl.astype(np.float32))


def _iota16() -> np.ndarray:
    return np.broadcast_to(
        np.arange(D, dtype=np.int16)[None, :], (P, D)
    ).copy()


def kernel(x: np.ndarray, **run_kwargs) -> np.ndarray:
    """Full-input entry point: x [4096, 8192] f32 -> scalar f32 loss."""
    x = np.ascontiguousarray(np.asarray(x, dtype=np.float32))
    assert x.shape == (ROWS, COLS)
    # rl[i] == COLS for every row iff x has no exact zeros; then the
    # validity mask is all-true and the count/mask passes are unneeded.
    fast = np.count_nonzero(x) == x.size
    if fast:
        nc = _get_nc("fast")
        in_maps = [
            {"x": x[i * SH_ROWS : (i + 1) * SH_ROWS]} for i in range(N_CORES)
        ]
        res = run_bass_kernel_spmd(nc, in_maps, list(range(N_CORES)), **run_kwargs)
        # y[p, t] = rowsum of |diff| for local row t*P + p; rl == COLS
        losses = np.concatenate(
            [
                res.results[i]["y"].T.reshape(-1).astype(np.float32)
                for i in range(N_CORES)
            ]
        ) / np.float32(COLS)
    else:
        nc = _get_nc("v2")
        it = _iota16()
        in_maps = [
            {"x": x[i * SH_ROWS : (i + 1) * SH_ROWS], "iota16": it}
            for i in range(N_CORES)
        ]
        res = run_bass_kernel_spmd(nc, in_maps, list(range(N_CORES)), **run_kwargs)
        losses = np.concatenate(
            [_losses_from_y(res.results[i]["y"]) for i in range(N_CORES)]
        )
    total = losses[1:].sum(dtype=np.float64) / float(ROWS)
    out = np.float32(total)
    if run_kwargs:
        kernel.last_results = res  # type: ignore[attr-defined]
    return out

